# revision 17
# baseline (speedup 1.0000x reference)
"""Causal single-head attention (B=4, S=2048, D=DK=1024) on 8 trn2 NeuronCores.

Sharding: data-parallel over batch x interleaved q-blocks. Core c handles
batch b=c//2, parity p=c%2, owning the 8 q-blocks {2j+p : j in 0..7} (128 rows
each). One uniform SPMD program runs on all 8 cores; per-core differences are
carried entirely by the input data (host-side column permutation + mask tiles).

Math per core (weight-folded to skip full-context K/V projections; W_QK =
W_Q W_K^T is folded on the host):
    G^T = W_QK^T X_q^T                [d, 1024]
    S   = G X_ctx^T   (causal window, compact 2-region layout)
    A   = softmax(S/32 with -1e9 mask pre-scale)
    P   = A X_ctx     (bf16)
    out = P W_V       (then scatter rows back on host)

Matmuls run as float32r (full PE rate at N>=256, ~tf32 precision); the A@X
stage runs bf16. Host pre-transposes X^T / W_K^T so no fp32 DMA transposes are
needed on-chip.
"""

import numpy as np
import ml_dtypes

B, S, D = 4, 2048, 1024
P = 128               # partitions
NJ = 8                # q-tiles per core
NCORES = 8
MASK_FILL = -1.0e9

_cache = {}


def _build_program():
    from contextlib import ExitStack
    import concourse.bass as bass
    import concourse.bacc as bacc
    import concourse.tile as tile
    import concourse.mybir as mybir
    from concourse import masks

    f32 = mybir.dt.float32
    f32r = mybir.dt.float32r
    bf16 = mybir.dt.bfloat16
    Exp = mybir.ActivationFunctionType.Exp
    AX = mybir.AxisListType.X
    ts = bass.ts

    nc = bacc.Bacc("TRN2", target_bir_lowering=False, debug=False,
                   enable_asserts=False)

    xct_d = nc.dram_tensor("xct", [D, S], f32r, kind="ExternalInput").ap()
    xc_d = nc.dram_tensor("xc", [S, D], bf16, kind="ExternalInput").ap()
    wqk_d = nc.dram_tensor("wqk", [D, D], f32r, kind="ExternalInput").ap()
    wv_d = nc.dram_tensor("wv", [D, D], f32r, kind="ExternalInput").ap()
    madd_d = nc.dram_tensor("madd", [NJ * P, 2 * P], f32,
                            kind="ExternalInput").ap()
    out_d = nc.dram_tensor("out", [NJ * P, D], f32, kind="ExternalOutput").ap()

    xct_r = xct_d.rearrange("(c p) k -> c p k", p=P)    # [8, 128, 2048]
    xc_r = xc_d.rearrange("(c p) d -> c p d", p=P)      # [16, 128, 1024]
    wqk_r = wqk_d.rearrange("(c p) n -> c p n", p=P)
    wv_r = wv_d.rearrange("(c p) n -> c p n", p=P)

    with tile.TileContext(nc) as tc, ExitStack() as es:
        # ---- persistent pools -------------------------------------------
        perm = es.enter_context(tc.tile_pool(name="perm", bufs=1))
        xct_sb = perm.tile([P, 8, S], f32r)        # X_ctx^T  64KB/part
        xc_sb = perm.tile([P, 16, D], bf16)       # X_ctx (perm rows) 32KB/part
        gt_sb = perm.tile([P, 8, 1024], f32r)      # G^T 32KB/part
        ident_b = perm.tile([P, P], bf16)
        ident_f = perm.tile([P, P], f32)

        masks.make_identity(nc, ident_b[:])
        masks.make_identity(nc, ident_f[:])

        # ---- phase G: G^T = (W_Q W_K^T)^T X_q^T -------------------------
        # W_QK is folded on the host; dc-outer accumulation into 8 live PSUM
        # banks so the first matmul only needs the first wqk/xct chunk. wv is
        # resident on the right-side stack and prefetches under this phase.
        wv_pool = tc.alloc_tile_pool(name="wv", bufs=1, side="right")
        wv_sb = wv_pool.tile([P, 8, 1024], f32r)
        # scores psum pool allocated ahead of G's pool so j=0's score matmuls
        # don't wait on the G-pool release barrier
        spsp = tc.alloc_tile_pool(name="sps", bufs=2, space="PSUM")

        with tc.tile_pool(name="wqk", bufs=1) as wqkp, \
             tc.tile_pool(name="pps", bufs=6, space="PSUM") as pps:
            wqk_sb = wqkp.tile([P, 8, 1024], f32r)
            for dc in range(8):
                nc.sync.dma_start(wqk_sb[:, dc, :], wqk_r[dc])
                nc.sync.dma_start(xct_sb[:, dc, 0:512], xct_r[dc, :, 0:512])
            for dc in range(8):
                nc.sync.dma_start(xct_sb[:, dc, 512:1024],
                                  xct_r[dc, :, 512:1024])
            # qh-outer: pass 0 needs only the first xct q-half from DRAM;
            # pass 1 then runs entirely on SBUF-resident data, covering the
            # window where phase-D inputs are still streaming in.
            for qh in (0, 512):
                psl = {dt_: pps.tile([P, 512], f32, tag="ps",
                                     name=f"psG{dt_}{qh}")
                       for dt_ in range(8)}
                for dc in range(8):
                    for dt_ in range(8):
                        nc.tensor.matmul(
                            psl[dt_][:], wqk_sb[:, dc, ts(dt_, P)],
                            xct_sb[:, dc, qh:qh + 512],
                            start=(dc == 0), stop=(dc == 7))
                for dt_ in range(8):
                    nc.vector.tensor_copy(gt_sb[:, dt_, qh:qh + 512],
                                          psl[dt_][:])

        # phase-D inputs, issued on the sync FIFO in first-use order:
        # scores j=0 region2 needs xct cols [1024:1536]; out j=0 needs wv;
        # P j needs xc position blocks {0..j, 8..8+j}.
        for dc in range(8):
            nc.sync.dma_start(xct_sb[:, dc, 1024:1536],
                              xct_r[dc, :, 1024:1536])
        for dc in range(8):
            nc.sync.dma_start(wv_sb[:, dc, :], wv_r[dc])
        for i in range(8):
            for kb in (i, 8 + i):
                nc.sync.dma_start(xc_sb[:, kb, :], xc_r[kb])
        for dc in range(8):
            nc.sync.dma_start(xct_sb[:, dc, 1536:2048],
                              xct_r[dc, :, 1536:2048])

        # ---- phase D: attention per q-tile ------------------------------
        with tc.tile_pool(name="work1", bufs=1) as work1, \
             tc.tile_pool(name="work2", bufs=2) as work2, \
             tc.tile_pool(name="stats", bufs=4) as statp, \
             tc.tile_pool(name="trp", bufs=2, space="PSUM") as trp, \
             tc.tile_pool(name="ppp", bufs=2, space="PSUM") as ppp, \
             tc.tile_pool(name="ops", bufs=2, space="PSUM") as opsp:
            for j in (0, 1, 2, 3, 4, 7, 5, 6):
                nk = 2 * j + 2          # 128-wide k-chunks this q-tile
                W = nk * P              # compact context width
                hw = (j + 1) * P        # per-region width

                madd_t = work2.tile([P, 2 * P], f32, tag="madd")
                nc.scalar.dma_start(madd_t[:], madd_d[ts(j, P), :])

                srow = work1.tile([P, 2048], f32, tag="srow", bufs=2)
                # two column regions: own q-blocks [0:1024), others [1024:2048)
                # segmented softmax: per-segment max during the copies, then
                # segmented exp so transposes can start before the whole row
                # is exponentiated.
                segs = []  # (dst_off, width, mask_col) per <=512-wide segment
                for ri, (base_src, base_dst) in enumerate(((0, 0), (1024, hw))):
                    for off in range(0, hw, 512):
                        w = min(512, hw - off)
                        ps = spsp.tile([P, 512], f32, tag="ps")
                        for dc in range(8):
                            nc.tensor.matmul(
                                ps[:, :w], gt_sb[:, dc, ts(j, P)],
                                xct_sb[:, dc,
                                       base_src + off:base_src + off + w],
                                start=(dc == 0), stop=(dc == 7))
                        dst = base_dst + off
                        nc.vector.tensor_copy(srow[:, dst:dst + w],
                                              ps[:, :w])
                        segs.append((dst, w, ri if off + w == hw else None))
                mxseg = statp.tile([P, 4], f32, tag="mxseg")
                for si, (dst, w, ri) in enumerate(segs):
                    if ri is not None:
                        # boundary chunk of region ri sits at this segment's
                        # tail: apply the additive causal mask before the max
                        chunk = ts(j, P) if ri == 0 else ts(2 * j + 1, P)
                        nc.vector.tensor_add(srow[:, chunk], srow[:, chunk],
                                             madd_t[:, ri * P:(ri + 1) * P])
                    nc.vector.reduce_max(mxseg[:, si:si + 1],
                                         srow[:, dst:dst + w], axis=AX)
                nmx = statp.tile([P, 1], f32, tag="nmx")
                nc.vector.reduce_max(nmx[:], mxseg[:, :len(segs)], axis=AX,
                                     negate=True)
                nc.scalar.mul(nmx[:], nmx[:], 1.0 / 32.0)
                seseg = statp.tile([P, 4], f32, tag="seseg")
                attn = work1.tile([P, 2048], bf16, tag="attn")
                for si, (dst, w, _) in enumerate(segs):
                    nc.scalar.activation(attn[:, dst:dst + w],
                                         srow[:, dst:dst + w], Exp,
                                         bias=nmx[:], scale=1.0 / 32.0,
                                         accum_out=seseg[:, si:si + 1])
                sumexp = statp.tile([P, 1], f32, tag="se")
                nc.vector.reduce_sum(sumexp[:], seseg[:, :len(segs)], axis=AX)
                rcp = statp.tile([P, 1], f32, tag="rcp")
                nc.vector.reciprocal(rcp[:], sumexp[:])

                attnT = work1.tile([P, 2048], bf16, tag="attnT")
                for c in range(nk):
                    tp = trp.tile([P, P], bf16, tag="tr")
                    nc.tensor.transpose(tp[:], attn[:, ts(c, P)], ident_b[:])
                    nc.vector.tensor_copy(attnT[:, ts(c, P)], tp[:])

                p_sb = work2.tile([P, 1024], f32, tag="p", bufs=1)
                for dh in (0, 512):
                    pp = ppp.tile([P, 512], f32, tag="pp")
                    for c in range(nk):
                        pos = c if c <= j else 8 + (c - j - 1)
                        nc.tensor.matmul(
                            pp[:], attnT[:, ts(c, P)],
                            xc_sb[:, pos, dh:dh + 512],
                            start=(c == 0), stop=(c == nk - 1))
                    nc.vector.tensor_copy(p_sb[:, dh:dh + 512], pp[:])

                pt_sb = work2.tile([P, 1024], f32r, tag="pt", bufs=1)
                for dc in range(8):
                    tp = trp.tile([P, P], f32, tag="tr")
                    nc.tensor.transpose(tp[:], p_sb[:, ts(dc, P)], ident_f[:])
                    nc.vector.tensor_copy(pt_sb[:, ts(dc, P)], tp[:])

                out_sb = work2.tile([P, 1024], f32, tag="out")
                for dvh in (0, 512):
                    op = opsp.tile([P, 512], f32, tag="op")
                    for dc in range(8):
                        nc.tensor.matmul(
                            op[:], pt_sb[:, ts(dc, P)],
                            wv_sb[:, dc, dvh:dvh + 512],
                            start=(dc == 0), stop=(dc == 7))
                    # normalize by softmax denominator during PSUM->SBUF copy
                    nc.scalar.activation(
                        out_sb[:, dvh:dvh + 512], op[:],
                        mybir.ActivationFunctionType.Copy, scale=rcp[:])
                nc.sync.dma_start(out_d[ts(j, P), :], out_sb[:])
        spsp.release()
        wv_pool.release()

    nc.compile()
    return nc


def _prep_inputs(sequence_repr, W_Q, W_K, W_V, mask):
    """Build the 8 per-core input dicts (host-side slicing/permutation)."""
    wqk = np.ascontiguousarray(W_Q @ W_K.T)
    in_maps = []
    meta = []
    for c in range(NCORES):
        b, par = divmod(c, 2)
        qblocks = [2 * j + par for j in range(NJ)]
        oblocks = [2 * j + 1 - par for j in range(NJ)]
        posblocks = qblocks + oblocks
        rows_perm = np.concatenate(
            [np.arange(g * P, (g + 1) * P) for g in posblocks])
        qrows = rows_perm[:NJ * P]
        xb = sequence_repr[b]
        xct = np.ascontiguousarray(xb.T[:, rows_perm])
        xc = np.ascontiguousarray(xb[rows_perm]).astype(ml_dtypes.bfloat16)
        madd = np.empty((NJ * P, 2 * P), np.float32)
        for j in range(NJ):
            g = 2 * j + par
            gb = 2 * j + 1 - par
            qr = slice((2 * j + par) * P, (2 * j + par) * P + P)
            madd[j * P:(j + 1) * P, 0:P] = np.where(
                mask[b, qr, g * P:(g + 1) * P], 0.0, MASK_FILL)
            madd[j * P:(j + 1) * P, P:2 * P] = np.where(
                mask[b, qr, gb * P:(gb + 1) * P], 0.0, MASK_FILL)
        in_maps.append({
            "xct": xct, "xc": xc,
            "wqk": wqk,
            "wv": np.ascontiguousarray(W_V),
            "madd": madd,
        })
        meta.append((b, qrows))
    return in_maps, meta


def run(sequence_repr, W_Q, W_K, W_V, mask, trace=False):
    from concourse.bass_utils import run_bass_kernel_spmd

    if "nc" not in _cache:
        _cache["nc"] = _build_program()
    nc = _cache["nc"]
    in_maps, meta = _prep_inputs(
        np.asarray(sequence_repr, np.float32), np.asarray(W_Q, np.float32),
        np.asarray(W_K, np.float32), np.asarray(W_V, np.float32),
        np.asarray(mask))
    res = run_bass_kernel_spmd(nc, in_maps, core_ids=list(range(NCORES)),
                               trace=trace)
    out = np.empty((B, S, D), np.float32)
    for c in range(NCORES):
        b, qrows = meta[c]
        out[b, qrows] = res.results[c]["out"]
    return out, res


def kernel(**inputs):
    out, _ = run(**inputs)
    return out
